# revision 2
# baseline (speedup 1.0000x reference)
"""Trainium2 Bass kernel for a MountainCar dynamics-model RNN.

Model (per batch element, T=500 steps, H=40):
    xn      = (x - MEAN_IN) / STD_IN                       # [T, 3]
    s2_{t+1} = tanh(xn_t[:2] @ Ks + xn_t[2:] @ Ka + bias + s2_t @ Kh)
    out     = clip((s2_T @ dense_w + dense_b) * STD_OUT + MEAN_OUT)

Device mapping (8 NeuronCores, batch-parallel, 4096 batch each):
  - THREE batch groups stacked on the partition dim (3 x H = 120 rows) so
    the ScalarE tanh — the throughput floor of this kernel — runs at
    120/128 lane utilization. Free dim = 1366 columns per group
    (3 x 1366 = 4098 >= 4096, last 2 columns padding).
  - bf16 state / weights / x: the PE streams 1 column/cycle for bf16
    (vs 2 for fp32/f32r), PSUM accumulation stays fp32, and tanh is
    evaluated in fp32 from PSUM. Measured end-to-end scale-relative
    error ~5e-3 (gate 2e-2).
  - per step: 3 x-projection matmuls (K=9 block-diag W3') + 3 recurrence
    matmuls (K=120 block-diag Kh) accumulate into one 3-bank PSUM tile;
    2 tanh ops (cols 0-1023, 1024-1365) write the next state with bias
    c' folded in. Ping-pong PSUM/state tiles give cross-step overlap:
    the x matmuls of step t+1 run under the tanh of step t.
  - input normalization is folded into W3'/c' host-side; x is
    pre-transposed to [T, 9, 1366] bf16 host-side, one DMA per step.
"""

import sys

sys.path.insert(0, "/opt/trn_rl_repo")

import ml_dtypes
import numpy as np

# ---------------------------------------------------------------- constants
B, T_FULL, F, H = 32768, 500, 3, 40
NCORES = 8
BS = B // NCORES          # 4096 batch per core
NG = 3                    # batch groups stacked on partitions
G = 1366                  # columns per group (3*1366 = 4098 >= 4096)
ROWS = NG * H             # 120 state rows
XR = NG * F               # 9 x rows
BANKS = [(0, 512), (512, 512), (1024, G - 1024)]      # matmul col splits
CHUNKS = [(0, 1024), (1024, G - 1024)]                # tanh col splits

MEAN_IN = np.array([-0.3, 0.0, 0.0], np.float32)
STD_IN = np.array([0.9, 0.07, 1.0], np.float32)
MEAN_OUT = np.array([-0.3, 0.0], np.float32)
STD_OUT = np.array([0.9, 0.07], np.float32)
MIN_POS, MAX_POS, MAX_SPEED = -1.2, 0.6, 0.07

BF16 = ml_dtypes.bfloat16


# ---------------------------------------------------------------- host prep
def _host_weights(kernel_state, kernel_hidden, kernel_action, bias,
                  dense_w, dense_b):
    """Fold normalization into the weights; build block-diag layouts."""
    w3 = np.vstack([kernel_state, kernel_action]).astype(np.float32)  # [3,40]
    w3p = w3 / STD_IN[:, None]
    cp = (bias[0] - (MEAN_IN / STD_IN) @ w3).astype(np.float32)       # [40]
    kh = kernel_hidden.astype(np.float32)

    wst = np.zeros((ROWS, ROWS), np.float32)      # recurrence lhsT
    wx = np.zeros((XR, ROWS), np.float32)         # x-projection lhsT
    c120 = np.zeros((ROWS, 1), np.float32)        # tanh bias
    dw120 = np.zeros((ROWS, 2 * NG), np.float32)  # endgame lhsT
    for g in range(NG):
        wst[H * g:H * g + H, H * g:H * g + H] = kh
        for f in range(F):
            wx[F * g + f, H * g:H * g + H] = w3p[f]
        c120[H * g:H * g + H, 0] = cp
        dw120[H * g:H * g + H, 2 * g:2 * g + 2] = dense_w * STD_OUT[None, :]

    dbp = (dense_b * STD_OUT + MEAN_OUT).astype(np.float32)           # [2]
    db6 = np.zeros((2 * NG, 1), np.float32)
    lo6 = np.zeros((2 * NG, 1), np.float32)
    hi6 = np.zeros((2 * NG, 1), np.float32)
    for g in range(NG):
        db6[2 * g + 0, 0], db6[2 * g + 1, 0] = dbp[0], dbp[1]
        lo6[2 * g + 0, 0], lo6[2 * g + 1, 0] = MIN_POS, -MAX_SPEED
        hi6[2 * g + 0, 0], hi6[2 * g + 1, 0] = MAX_POS, MAX_SPEED
    return dict(wst=wst.astype(BF16), wx=wx.astype(BF16), c120=c120,
                dw120=dw120.astype(BF16), db6=db6, lo6=lo6, hi6=hi6)


def _host_x_shard(x_core, t_steps):
    """[BS, T, 3] -> [T, 9, G] bf16, row 3*g+f, col j, batch b = G*g + j."""
    xp = np.zeros((NG * G, t_steps, F), np.float32)
    xp[:x_core.shape[0]] = x_core
    xt = xp.reshape(NG, G, t_steps, F).transpose(2, 0, 3, 1)  # [T,NG,F,G]
    return np.ascontiguousarray(xt.reshape(t_steps, XR, G).astype(BF16))


# ---------------------------------------------------------------- bass prog
def build_program(t_steps=T_FULL):
    import concourse.bass as bass
    import concourse.tile as tile
    from concourse import bacc, mybir
    from concourse._compat import with_exitstack
    from contextlib import ExitStack

    f32 = mybir.dt.float32
    bf16 = mybir.dt.bfloat16
    nc = bacc.Bacc("TRN2", target_bir_lowering=False, debug=False,
                   enable_asserts=True, num_devices=NCORES)

    ins = {}
    for name, shape, dt in [("xt", [t_steps, XR, G], bf16),
                            ("wst", [ROWS, ROWS], bf16),
                            ("wx", [XR, ROWS], bf16),
                            ("c120", [ROWS, 1], f32),
                            ("dw120", [ROWS, 2 * NG], bf16),
                            ("db6", [2 * NG, 1], f32),
                            ("lo6", [2 * NG, 1], f32),
                            ("hi6", [2 * NG, 1], f32)]:
        ins[name] = nc.dram_tensor(name, shape, dt, kind="ExternalInput").ap()
    out_d = nc.dram_tensor("out", [2 * NG, G], f32, kind="ExternalOutput").ap()

    Tanh = mybir.ActivationFunctionType.Tanh
    Alu = mybir.AluOpType

    @with_exitstack
    def body(ctx, tc):
        nc = tc.nc
        singles = ctx.enter_context(tc.tile_pool(name="singles", bufs=1))
        xpool = ctx.enter_context(tc.tile_pool(name="xp", bufs=4))
        spool = ctx.enter_context(tc.tile_pool(name="state", bufs=1))
        pspool = ctx.enter_context(tc.tile_pool(name="ps", bufs=1,
                                                space="PSUM"))

        wst_t = singles.tile([ROWS, ROWS], bf16)
        nc.sync.dma_start(out=wst_t[:], in_=ins["wst"])
        wx_t = singles.tile([XR, ROWS], bf16)
        nc.sync.dma_start(out=wx_t[:], in_=ins["wx"])
        c_t = singles.tile([ROWS, 1], f32)
        nc.sync.dma_start(out=c_t[:], in_=ins["c120"])
        dw_t = singles.tile([ROWS, 2 * NG], bf16)
        nc.sync.dma_start(out=dw_t[:], in_=ins["dw120"])
        db_t = singles.tile([2 * NG, 1], f32)
        nc.sync.dma_start(out=db_t[:], in_=ins["db6"])
        lo_t = singles.tile([2 * NG, 1], f32)
        nc.sync.dma_start(out=lo_t[:], in_=ins["lo6"])
        hi_t = singles.tile([2 * NG, 1], f32)
        nc.sync.dma_start(out=hi_t[:], in_=ins["hi6"])

        S = [spool.tile([ROWS, G], bf16, tag=f"s{i}", name=f"s{i}")
             for i in range(2)]
        # 4 banks each (pad to 2048 cols) so both tiles stay bank-aligned
        PS = [pspool.tile([ROWS, G], f32, tag=f"ps{i}", name=f"ps{i}",
                          padded_shape=[ROWS, 2048])
              for i in range(2)]

        def step(t):
            ps = PS[t % 2]
            cur, nxt = S[t % 2], S[(t + 1) % 2]
            first = (t == 0)
            xb = xpool.tile([XR, G], bf16, name="xb", tag="xb")
            nc.sync.dma_start(out=xb[:], in_=ins["xt"][t])
            # x-projection first: it has no dependence on the previous
            # tanh, so it runs under step t-1's activations.
            for c0, n in BANKS:
                nc.tensor.matmul(
                    ps[0:ROWS, c0:c0 + n],
                    lhsT=wx_t[:],
                    rhs=xb[0:XR, c0:c0 + n],
                    start=True, stop=first,
                    skip_group_check=True,
                    tile_position=(0, 0))
            if not first:
                for c0, n in BANKS:
                    nc.tensor.matmul(
                        ps[0:ROWS, c0:c0 + n],
                        lhsT=wst_t[:],
                        rhs=cur[0:ROWS, c0:c0 + n],
                        start=False, stop=True,
                        skip_group_check=True,
                        tile_position=(0, 0))
            # s2' = tanh(psum + c'); two chunks so the next step's
            # matmuls start as soon as the first chunk lands.
            for c0, n in CHUNKS:
                nc.scalar.activation(nxt[0:ROWS, c0:c0 + n],
                                     ps[0:ROWS, c0:c0 + n], Tanh,
                                     bias=c_t[0:ROWS, 0:1])

        for t in range(t_steps):
            step(t)

        # endgame: out = clip((s2 @ dw') + db'), groups stacked as 6 rows
        sf = S[t_steps % 2]
        pse = PS[t_steps % 2]
        for c0, n in BANKS:
            nc.tensor.matmul(
                pse[0:2 * NG, c0:c0 + n],
                lhsT=dw_t[:],
                rhs=sf[0:ROWS, c0:c0 + n],
                start=True, stop=True,
                skip_group_check=True,
                tile_position=(0, 0))
        ob = singles.tile([2 * NG, G], f32)
        nc.vector.tensor_scalar(ob[:], pse[0:2 * NG, 0:G],
                                scalar1=db_t[0:2 * NG, 0:1],
                                scalar2=hi_t[0:2 * NG, 0:1],
                                op0=Alu.add, op1=Alu.min)
        nc.vector.tensor_scalar(ob[:], ob[:],
                                scalar1=lo_t[0:2 * NG, 0:1], scalar2=None,
                                op0=Alu.max)
        nc.sync.dma_start(out=out_d, in_=ob[:])

    import concourse.tile as tile_mod
    with tile_mod.TileContext(nc) as tc:
        body(tc)
    nc.compile()
    return nc


# ---------------------------------------------------------------- execution
def _assemble_out(out_core):
    """[6, G] device layout -> [BS, 2]: row 2g+c, col j <-> batch G*g+j."""
    o = np.asarray(out_core, dtype=np.float32).reshape(NG, 2, G)
    return np.ascontiguousarray(o.transpose(0, 2, 1).reshape(NG * G, 2)[:BS])


def _make_in_maps(x, weights, t_steps):
    in_maps = []
    for c in range(NCORES):
        m = dict(weights)
        m["xt"] = _host_x_shard(
            np.asarray(x[c * BS:(c + 1) * BS, :t_steps], dtype=np.float32),
            t_steps)
        in_maps.append(m)
    return in_maps


def run(x, kernel_state, kernel_hidden, kernel_action, bias, dense_w,
        dense_b, t_steps=T_FULL, trace=False, nc=None):
    from concourse.bass_utils import run_bass_kernel_spmd
    if nc is None:
        nc = build_program(t_steps)
    weights = _host_weights(kernel_state, kernel_hidden, kernel_action,
                            bias, dense_w, dense_b)
    in_maps = _make_in_maps(x, weights, t_steps)
    res = run_bass_kernel_spmd(nc, in_maps, core_ids=list(range(NCORES)),
                               trace=trace)
    outs = [_assemble_out(res.results[c]["out"]) for c in range(NCORES)]
    return np.concatenate(outs, axis=0), res


def kernel(x, kernel_state, kernel_hidden, kernel_action, bias, dense_w,
           dense_b):
    out, _ = run(np.asarray(x), np.asarray(kernel_state),
                 np.asarray(kernel_hidden), np.asarray(kernel_action),
                 np.asarray(bias), np.asarray(dense_w), np.asarray(dense_b))
    return out


# revision 8
# speedup vs baseline: 1.0554x; 1.0554x over previous
"""Trainium2 Bass kernel for a MountainCar dynamics-model RNN.

Model (per batch element, T=500 steps, H=40):
    xn      = (x - MEAN_IN) / STD_IN                       # [T, 3]
    s2_{t+1} = tanh(xn_t[:2] @ Ks + xn_t[2:] @ Ka + bias + s2_t @ Kh)
    out     = clip((s2_T @ dense_w + dense_b) * STD_OUT + MEAN_OUT)

Device mapping (8 NeuronCores, batch-parallel, 4096 batch each):
  - THREE batch groups stacked on the partition dim (3 x H = 120 rows) so
    the ScalarE tanh — the throughput floor of this kernel — runs at
    120/128 lane utilization. Free dim = 1366 columns per group
    (3 x 1366 = 4098 >= 4096, last 2 columns padding).
  - bf16 state / weights / x: the PE streams 1 column/cycle for bf16
    (vs 2 for fp32/f32r), PSUM accumulation stays fp32, and tanh is
    evaluated in fp32 from PSUM. Measured end-to-end scale-relative
    error ~5e-3 (gate 2e-2).
  - per step: 3 x-projection matmuls (K=9 block-diag W3') + 3 recurrence
    matmuls (K=120 block-diag Kh) accumulate into one 3-bank PSUM tile;
    2 tanh ops (cols 0-1023, 1024-1365) write the next state with bias
    c' folded in. Ping-pong PSUM/state tiles give cross-step overlap:
    the x matmuls of step t+1 run under the tanh of step t.
  - input normalization is folded into W3'/c' host-side; x is
    pre-transposed to [T, 9, 1366] bf16 host-side, one DMA per step.
"""

import sys

sys.path.insert(0, "/opt/trn_rl_repo")

import ml_dtypes
import numpy as np

# ---------------------------------------------------------------- constants
B, T_FULL, F, H = 32768, 500, 3, 40
NCORES = 8
BS = B // NCORES          # 4096 batch per core
NG = 3                    # batch groups stacked on partitions
G = 1366                  # columns per group (3*1366 = 4098 >= 4096)
ROWS = NG * H             # 120 state rows
XR = NG * F               # 9 x rows
BANKS = [(0, 512), (512, 512), (1024, G - 1024)]      # matmul col splits
CHUNKS = [(0, 1024), (1024, G - 1024)]                # tanh col splits

MEAN_IN = np.array([-0.3, 0.0, 0.0], np.float32)
STD_IN = np.array([0.9, 0.07, 1.0], np.float32)
MEAN_OUT = np.array([-0.3, 0.0], np.float32)
STD_OUT = np.array([0.9, 0.07], np.float32)
MIN_POS, MAX_POS, MAX_SPEED = -1.2, 0.6, 0.07

BF16 = ml_dtypes.bfloat16


# ---------------------------------------------------------------- host prep
def _host_weights(kernel_state, kernel_hidden, kernel_action, bias,
                  dense_w, dense_b):
    """Fold normalization into the weights; build block-diag layouts."""
    w3 = np.vstack([kernel_state, kernel_action]).astype(np.float32)  # [3,40]
    w3p = w3 / STD_IN[:, None]
    cp = (bias[0] - (MEAN_IN / STD_IN) @ w3).astype(np.float32)       # [40]
    kh = kernel_hidden.astype(np.float32)

    # lhsT free dim padded to 128 so the compiler's Fast Weight Load kicks
    # in (requires NumWeights==128, non-fp32); psum rows 120-127 get zeros.
    wst = np.zeros((ROWS, 128), np.float32)       # recurrence lhsT
    wx = np.zeros((XR, 128), np.float32)          # x-projection lhsT
    c120 = np.zeros((ROWS, 1), np.float32)        # tanh bias
    dw120 = np.zeros((ROWS, 2 * NG), np.float32)  # endgame lhsT
    for g in range(NG):
        wst[H * g:H * g + H, H * g:H * g + H] = kh
        for f in range(F):
            wx[F * g + f, H * g:H * g + H] = w3p[f]
        c120[H * g:H * g + H, 0] = cp
        dw120[H * g:H * g + H, 2 * g:2 * g + 2] = dense_w * STD_OUT[None, :]
    # x lhsT replicated at PE row homes {0, 32, 64} so the three
    # x-projection matmuls (one per bank) run concurrently on the array.
    wx3 = np.zeros((64 + XR, 128), np.float32)
    for b in range(len(BANKS)):
        wx3[32 * b:32 * b + XR] = wx

    dbp = (dense_b * STD_OUT + MEAN_OUT).astype(np.float32)           # [2]
    db6 = np.zeros((2 * NG, 1), np.float32)
    lo6 = np.zeros((2 * NG, 1), np.float32)
    hi6 = np.zeros((2 * NG, 1), np.float32)
    for g in range(NG):
        db6[2 * g + 0, 0], db6[2 * g + 1, 0] = dbp[0], dbp[1]
        lo6[2 * g + 0, 0], lo6[2 * g + 1, 0] = MIN_POS, -MAX_SPEED
        hi6[2 * g + 0, 0], hi6[2 * g + 1, 0] = MAX_POS, MAX_SPEED
    return dict(wst=wst.astype(BF16), wx3=wx3.astype(BF16), c120=c120,
                dw120=dw120.astype(BF16), db6=db6, lo6=lo6, hi6=hi6)


def _host_x_shard(x_core, t_steps):
    """[BS, T, 3] -> [T, 9, G] bf16, row 3*g+f, col j, batch b = G*g + j."""
    xp = np.zeros((NG * G, t_steps, F), np.float32)
    xp[:x_core.shape[0]] = x_core
    xt = xp.reshape(NG, G, t_steps, F).transpose(2, 0, 3, 1)  # [T,NG,F,G]
    return np.ascontiguousarray(xt.reshape(t_steps, XR, G).astype(BF16))


# ---------------------------------------------------------------- bass prog
def build_program(t_steps=T_FULL):
    import concourse.bass as bass
    import concourse.tile as tile
    from concourse import bacc, mybir
    from concourse._compat import with_exitstack
    from contextlib import ExitStack

    f32 = mybir.dt.float32
    bf16 = mybir.dt.bfloat16
    nc = bacc.Bacc("TRN2", target_bir_lowering=False, debug=False,
                   enable_asserts=True, num_devices=NCORES)

    ins = {}
    for name, shape, dt in [("xt", [t_steps, XR, G], bf16),
                            ("wst", [ROWS, 128], bf16),
                            ("wx3", [64 + XR, 128], bf16),
                            ("c120", [ROWS, 1], f32),
                            ("dw120", [ROWS, 2 * NG], bf16),
                            ("db6", [2 * NG, 1], f32),
                            ("lo6", [2 * NG, 1], f32),
                            ("hi6", [2 * NG, 1], f32)]:
        ins[name] = nc.dram_tensor(name, shape, dt, kind="ExternalInput").ap()
    out_d = nc.dram_tensor("out", [2 * NG, G], f32, kind="ExternalOutput").ap()

    Tanh = mybir.ActivationFunctionType.Tanh
    Alu = mybir.AluOpType

    @with_exitstack
    def body(ctx, tc):
        nc = tc.nc
        singles = ctx.enter_context(tc.tile_pool(name="singles", bufs=1))
        xpool = ctx.enter_context(tc.tile_pool(name="xp", bufs=4))
        spool = ctx.enter_context(tc.tile_pool(name="state", bufs=1))
        pspool = ctx.enter_context(tc.tile_pool(name="ps", bufs=1,
                                                space="PSUM"))

        wst_t = singles.tile([ROWS, 128], bf16)
        nc.sync.dma_start(out=wst_t[:], in_=ins["wst"])
        wx_t = singles.tile([64 + XR, 128], bf16)
        nc.sync.dma_start(out=wx_t[:], in_=ins["wx3"])
        c_t = singles.tile([ROWS, 1], f32)
        nc.sync.dma_start(out=c_t[:], in_=ins["c120"])
        dw_t = singles.tile([ROWS, 2 * NG], bf16)
        nc.sync.dma_start(out=dw_t[:], in_=ins["dw120"])
        db_t = singles.tile([2 * NG, 1], f32)
        nc.sync.dma_start(out=db_t[:], in_=ins["db6"])
        lo_t = singles.tile([2 * NG, 1], f32)
        nc.sync.dma_start(out=lo_t[:], in_=ins["lo6"])
        hi_t = singles.tile([2 * NG, 1], f32)
        nc.sync.dma_start(out=hi_t[:], in_=ins["hi6"])

        S = [spool.tile([ROWS, G], bf16, tag=f"s{i}", name=f"s{i}")
             for i in range(2)]
        # 4 banks each (pad to 2048 cols) so both tiles stay bank-aligned
        PS = [pspool.tile([128, G], f32, tag=f"ps{i}", name=f"ps{i}",
                          padded_shape=[128, 2048])
              for i in range(2)]

        def step(t):
            ps = PS[t % 2]
            cur, nxt = S[t % 2], S[(t + 1) % 2]
            first = (t == 0)
            xb = xpool.tile([64 + XR, G], bf16, name="xb", tag="xb")
            for b, (c0, n) in enumerate(BANKS):
                nc.sync.dma_start(out=xb[32 * b:32 * b + XR, c0:c0 + n],
                                  in_=ins["xt"][t, :, c0:c0 + n])
            # x-projection first: it has no dependence on the previous
            # tanh, so it runs under step t-1's activations; the three
            # banks sit at PE row homes {0,32,64} and run concurrently.
            for b, (c0, n) in enumerate(BANKS):
                nc.tensor.matmul(
                    ps[0:128, c0:c0 + n],
                    lhsT=wx_t[32 * b:32 * b + XR, :],
                    rhs=xb[32 * b:32 * b + XR, c0:c0 + n],
                    start=True, stop=first,
                    skip_group_check=True,
                    tile_position=(32 * b, 0))
            if not first:
                for c0, n in BANKS:
                    nc.tensor.matmul(
                        ps[0:128, c0:c0 + n],
                        lhsT=wst_t[:],
                        rhs=cur[0:ROWS, c0:c0 + n],
                        start=False, stop=True,
                        skip_group_check=True,
                        tile_position=(0, 0))
            # s2' = tanh(psum + c'); two chunks so the next step's
            # matmuls start as soon as the first chunk lands.
            for c0, n in CHUNKS:
                nc.scalar.activation(nxt[0:ROWS, c0:c0 + n],
                                     ps[0:ROWS, c0:c0 + n], Tanh,
                                     bias=c_t[0:ROWS, 0:1])

        for t in range(t_steps):
            step(t)

        # endgame: out = clip((s2 @ dw') + db'), groups stacked as 6 rows
        sf = S[t_steps % 2]
        pse = PS[t_steps % 2]
        for c0, n in BANKS:
            nc.tensor.matmul(
                pse[0:2 * NG, c0:c0 + n],
                lhsT=dw_t[:],
                rhs=sf[0:ROWS, c0:c0 + n],
                start=True, stop=True,
                skip_group_check=True,
                tile_position=(0, 0))
        ob = singles.tile([2 * NG, G], f32)
        nc.vector.tensor_scalar(ob[:], pse[0:2 * NG, 0:G],
                                scalar1=db_t[0:2 * NG, 0:1],
                                scalar2=hi_t[0:2 * NG, 0:1],
                                op0=Alu.add, op1=Alu.min)
        nc.vector.tensor_scalar(ob[:], ob[:],
                                scalar1=lo_t[0:2 * NG, 0:1], scalar2=None,
                                op0=Alu.max)
        nc.sync.dma_start(out=out_d, in_=ob[:])

    import concourse.tile as tile_mod
    with tile_mod.TileContext(nc) as tc:
        body(tc)
    nc.compile()
    return nc


# ---------------------------------------------------------------- execution
def _assemble_out(out_core):
    """[6, G] device layout -> [BS, 2]: row 2g+c, col j <-> batch G*g+j."""
    o = np.asarray(out_core, dtype=np.float32).reshape(NG, 2, G)
    return np.ascontiguousarray(o.transpose(0, 2, 1).reshape(NG * G, 2)[:BS])


def _make_in_maps(x, weights, t_steps):
    in_maps = []
    for c in range(NCORES):
        m = dict(weights)
        m["xt"] = _host_x_shard(
            np.asarray(x[c * BS:(c + 1) * BS, :t_steps], dtype=np.float32),
            t_steps)
        in_maps.append(m)
    return in_maps


def run(x, kernel_state, kernel_hidden, kernel_action, bias, dense_w,
        dense_b, t_steps=T_FULL, trace=False, nc=None):
    from concourse.bass_utils import run_bass_kernel_spmd
    if nc is None:
        nc = build_program(t_steps)
    weights = _host_weights(kernel_state, kernel_hidden, kernel_action,
                            bias, dense_w, dense_b)
    in_maps = _make_in_maps(x, weights, t_steps)
    res = run_bass_kernel_spmd(nc, in_maps, core_ids=list(range(NCORES)),
                               trace=trace)
    outs = [_assemble_out(res.results[c]["out"]) for c in range(NCORES)]
    return np.concatenate(outs, axis=0), res


def kernel(x, kernel_state, kernel_hidden, kernel_action, bias, dense_w,
           dense_b):
    out, _ = run(np.asarray(x), np.asarray(kernel_state),
                 np.asarray(kernel_hidden), np.asarray(kernel_action),
                 np.asarray(bias), np.asarray(dense_w), np.asarray(dense_b))
    return out
